# revision 25
# baseline (speedup 1.0000x reference)
"""Causal attention (B=4, S=2048, D=1024, fp32 in/out) on 8 Trainium2 cores.

Sharding: core c = (batch b = c//2, variant h = c%2). Queries are split at
64-row granularity: global 64-row chunk g (g=0..31 per batch) goes to
variant g%2, laid out in ascending order, so core column x maps to global
query row 128*(x//64) + 64*h + x%64.

This interleave makes the kernel's causal structure variant-INDEPENDENT:
  * scores strip for key tile kt covers exactly columns [64*kt, 1024) --
    68 (128x128x1024) tile-equivalents per core, the tile-granular minimum.
  * AV "slot" t = columns [128*t, 128*t+128) needs key tiles 0..2t+1
    (CNT = 2t+2 for every core) -- 72 tile-equivalents.
  * the causal mask reduces to ONE kt-independent [128, 64] tile applied to
    the first 64 columns of every strip (the diagonal chunk).

Numerics: Q^T and K^T are stored as fp8 e4m3 (q/k values are O(1): std
0.58, |x| < 6, well inside e4m3 range; quantization adds ~4% rms score
noise -> ~1.2% output error, comfortably under the 2e-2 budget) and the
scores matmul runs in DoubleRow perf mode: contraction 256 per matmul via
et-pair 3D APs, halving scores PE time.  The softmax 1/sqrt(D) scaling
moves into the exp (ScalarE activation scale=1/32) so q stays O(1) for
fp8.  V / P^T / AV stay bf16 (fp8 V would put ~2.6% directly on the
output).  Projections run in bf16; PSUM accumulation is fp32 throughout.

K/V are not recomputed per core: core (b, h) projects K^T/V only for its
own key half, and the pair exchanges halves with AllGathers over replica
groups [[0,1],[2,3],[4,5],[6,7]] through DRAM bounce buffers.  The CC
stream is the scarce resource (its init barrier alone costs ~40us of
kernel time and each 1MB gather ~11-16us), so: a tiny warm-up collective
fires as the FIRST gpsimd instruction to absorb the barrier, gathers are
split in halves triggered straight after each projection half's bounce
store (fp8 halves the K bytes), and the DMA rings are ordered so a bounce
store never queues behind low-urgency input loads.

Phase B per key tile kt: S^T strip [128 keys x (16-kt)*64 q] accumulated
over 4 DoubleRow e-pairs in PSUM; DVE adds the diagonal mask in-place in
PSUM; ScalarE exps straight from PSUM (scale=1/32) into per-slot bf16 P^T
tiles.  After strip 2t+1, slot t's AV runs: O = P^T.T V accumulated over
its 2t+2 key tiles with a fused ones-matmul row-sum, normalized by
1/rowsum on eviction (ScalarE Copy with scale=AP).

No max-subtraction in softmax: logits/32 have std ~0.33; masked entries
get -3.2e5 (pre-scale) -> exp underflows to 0.  ~24 dummy matmuls at t=0
warm the PE HAM clock gate while the input DMAs stream.
"""

import numpy as np
from contextlib import ExitStack

import ml_dtypes

import concourse.bass as bass
import concourse.tile as tile
from concourse import bacc, mybir
from concourse.bass_utils import run_bass_kernel_spmd

P = 128
B, S, D = 4, 2048, 1024
NCORES = 8
DT = D // P      # 8 contraction tiles
ST = S // P      # 16 key tiles (global)
SLOC = S // 2    # 1024 local keys per core
ET = D // P      # 8 output-feature tiles
QLOC = 1024      # query rows per core
NSLOT = 8        # AV slots of 128 query columns
CNT = tuple(2 * t + 2 for t in range(NSLOT))   # key tiles per slot
NEG = -320000.0  # pre-scale mask; /32 in the exp -> -1e4
NWARM = 14       # HAM warm-up matmuls

F32 = mybir.dt.float32
BF16 = mybir.dt.bfloat16
FP8 = mybir.dt.float8e4
FP8V = mybir.dt.float8e3   # e3m4 for the V exchange: 4-bit mantissa, range +-15.5
DR = mybir.MatmulPerfMode.DoubleRow

REPLICA_GROUPS = [[0, 1], [2, 3], [4, 5], [6, 7]]


def _chunks(width, step=512):
    out = []
    c0 = 0
    while c0 < width:
        out.append((c0, min(step, width - c0)))
        c0 += out[-1][1]
    return out


def _build():
    nc = bacc.Bacc("TRN2", target_bir_lowering=False, debug=False,
                   num_devices=NCORES)
    xt_in = nc.dram_tensor("xt", [D, SLOC], BF16, kind="ExternalInput").ap()
    xqt_in = nc.dram_tensor("xqt", [D, QLOC], BF16, kind="ExternalInput").ap()
    wq_in = nc.dram_tensor("wq", [D, D], BF16, kind="ExternalInput").ap()
    wk_in = nc.dram_tensor("wk", [D, D], BF16, kind="ExternalInput").ap()
    wv_in = nc.dram_tensor("wv", [D, D], BF16, kind="ExternalInput").ap()
    mask_in = nc.dram_tensor("mask", [P, 64], BF16, kind="ExternalInput").ap()
    out = nc.dram_tensor("out", [QLOC, D], F32, kind="ExternalOutput").ap()

    with tile.TileContext(nc) as tc, ExitStack() as ctx:
        persist = ctx.enter_context(tc.tile_pool(name="persist", bufs=1))
        kT = persist.tile([P, ET, S], FP8, tag="kT")       # K^T [e%128, et, key]
        qT = persist.tile([P, ET, QLOC], FP8, tag="qT")    # Q^T [e%128, et, q]
        v_sb = persist.tile([P, ST, D], BF16, tag="v")     # V   [k%128, kt, e]
        ones = persist.tile([P, 1], BF16, tag="ones")
        mask = persist.tile([P, 64], BF16, tag="mask")
        warm = persist.tile([P, 512], BF16, tag="warm")
        nc.gpsimd.memset(ones[:], 1.0)
        nc.gpsimd.memset(warm[:], 0.25)

        _emit_body(nc, tc, xt_in, xqt_in, wq_in, wk_in, wv_in, mask_in, out,
                   kT, qT, v_sb, ones, mask, warm)
    nc.compile()
    return nc


def _emit_body(nc, tc, xt_in, xqt_in, wq_in, wk_in, wv_in, mask_in, out,
               kT, qT, v_sb, ones, mask, warm):
    # ---------------- Phase A : projections + KV exchange ----------------
    with ExitStack() as pa:
        xp = pa.enter_context(tc.tile_pool(name="xp", bufs=1))
        dp = pa.enter_context(tc.tile_pool(name="dp", bufs=1, space="DRAM"))
        psA = pa.enter_context(tc.tile_pool(name="psA", bufs=8, space="PSUM"))

        # Tiny warm-up collective FIRST on the gpsimd queue: the CC stream's
        # init barrier (an all-core rendezvous, ~20us after ~15us of core
        # skew) starts at the first trigger, so trigger it at t~1us.
        warm_in = dp.tile([P, 8], BF16, tag="warm_in")
        warm_out = dp.tile([2 * P, 8], BF16, tag="warm_out")
        nc.gpsimd.collective_compute(
            "AllGather", mybir.AluOpType.bypass,
            replica_groups=REPLICA_GROUPS,
            ins=[warm_in.opt()], outs=[warm_out.opt()])

        # PE warm-up: keeps the HAM activity window busy from t~1us so the
        # first real matmul (waiting on wk/xt DMA) already runs at 2.4GHz.
        psw = psA.tile([P, 512], F32, tag="ps", name="psw")
        for _ in range(NWARM):
            nc.tensor.matmul(psw[:], lhsT=warm[:, 0:P], rhs=warm[:],
                             start=True, stop=True)

        xt = xp.tile([P, DT, SLOC], BF16, tag="xt")
        wq_t = xp.tile([P, DT, D], BF16, tag="wq")
        wk_t = xp.tile([P, DT, D], BF16, tag="wk")
        wv_t = xp.tile([P, DT, D], BF16, tag="wv")
        xqt = xp.tile([P, DT, QLOC], BF16, tag="xqt")
        klocal = xp.tile([P, ET, SLOC], FP8, tag="klocal")
        vlocal = xp.tile([P, ST // 2, D], FP8V, tag="vlocal")
        kbounce = dp.tile([D, SLOC], FP8, tag="kbounce")
        kgather = dp.tile([2 * D, SLOC], FP8, tag="kgather")
        # V is exchanged split by OUTPUT COLUMNS (e-halves), not key tiles:
        # AV output cols [0,512) need only gather A, cols [512,1024) only
        # gather B, so the AV phase splits into two column passes that track
        # the two gathers' arrival with no stall.
        vbounces = [dp.tile([SLOC, 512], FP8V, tag=f"vbounce{i}",
                            name=f"vbounce{i}") for i in range(2)]
        vgathers = [dp.tile([S, 512], FP8V, tag=f"vgather{i}",
                            name=f"vgather{i}") for i in range(2)]

        # Input DMAs.  K proj needs wk+xt slices in dt order first: stream
        # them on the two HW-DGE rings in parallel; the rest follow in
        # urgency order.  The bounce stores are emitted between input loads
        # at the points where their data is ready, so they never wait
        # behind a low-urgency input transfer.
        for dt in range(DT):
            nc.sync.dma_start(wk_t[:, dt, :], wk_in[dt * P:(dt + 1) * P, :])
            nc.scalar.dma_start(xt[:, dt, :], xt_in[dt * P:(dt + 1) * P, :])
        nc.gpsimd.dma_start(mask[:], mask_in[:, :])

        def _kproj_half(half, dt_outer):
            groups = [(et, kc) for et in range(half * 4, half * 4 + 4)
                      for kc in range(2)]
            pss = [psA.tile([P, 512], F32, tag="ps", name="ps")
                   for _ in groups]
            if dt_outer:
                # matmuls start as soon as the first wk/xt slices land
                for dt in range(DT):
                    for gi, (et, kc) in enumerate(groups):
                        nc.tensor.matmul(
                            pss[gi][:], lhsT=wk_t[:, dt, et * P:(et + 1) * P],
                            rhs=xt[:, dt, kc * 512:(kc + 1) * 512],
                            start=(dt == 0), stop=(dt == DT - 1))
                for gi, (et, kc) in enumerate(groups):
                    nc.vector.tensor_copy(
                        klocal[:, et, kc * 512:(kc + 1) * 512], pss[gi][:])
            else:
                # dt-inner: each group finishes early so its eviction
                # overlaps the remaining groups' matmuls.
                for gi, (et, kc) in enumerate(groups):
                    for dt in range(DT):
                        nc.tensor.matmul(
                            pss[gi][:], lhsT=wk_t[:, dt, et * P:(et + 1) * P],
                            rhs=xt[:, dt, kc * 512:(kc + 1) * 512],
                            start=(dt == 0), stop=(dt == DT - 1))
                    nc.vector.tensor_copy(
                        klocal[:, et, kc * 512:(kc + 1) * 512], pss[gi][:])
            et0 = half * 4
            qeng = nc.sync if half == 0 else nc.scalar
            qeng.dma_start(
                kbounce[et0 * P:(et0 + 4) * P, :].rearrange(
                    "(et p) k -> p et k", p=P),
                klocal[:, et0:et0 + 4, :])
            if half == 1:
                # Single 1MB fp8 gather for all of K: fewer ops on the
                # serial CC stream pulls both V gathers earlier.
                nc.gpsimd.collective_compute(
                    "AllGather", mybir.AluOpType.bypass,
                    replica_groups=REPLICA_GROUPS,
                    ins=[kbounce.opt()], outs=[kgather.opt()])

        def _vproj_echalf(ec):
            # One e-column half of V for ALL 8 local key tiles: 8 groups,
            # dt-inner so evictions (and the bounce store) pipeline.
            pss = [psA.tile([P, 512], F32, tag="ps", name="ps")
                   for _ in range(8)]
            for st in range(8):
                for dt in range(DT):
                    nc.tensor.matmul(
                        pss[st][:], lhsT=xt[:, dt, st * P:(st + 1) * P],
                        rhs=wv_t[:, dt, ec * 512:(ec + 1) * 512],
                        start=(dt == 0), stop=(dt == DT - 1))
                nc.vector.tensor_copy(
                    vlocal[:, st, ec * 512:(ec + 1) * 512], pss[st][:])
            qeng = nc.sync if ec == 0 else nc.scalar
            qeng.dma_start(
                vbounces[ec].rearrange("(st p) e -> p st e", p=P),
                vlocal[:, :, ec * 512:(ec + 1) * 512])
            nc.gpsimd.collective_compute(
                "AllGather", mybir.AluOpType.bypass,
                replica_groups=REPLICA_GROUPS,
                ins=[vbounces[ec].opt()], outs=[vgathers[ec].opt()])

        # K^T_loc[et, k] = sum_d Wk[d, et].T X_loc^T[d, k], evicted to fp8.
        _kproj_half(0, dt_outer=True)
        # sync ring: kbounce-A store just went in; wq follows (Q proj needs
        # it only at ~70us).  scalar ring: wv next (V proj needs it at ~40).
        for dt in range(DT):
            nc.sync.dma_start(wq_t[:, dt, :], wq_in[dt * P:(dt + 1) * P, :])
            nc.scalar.dma_start(wv_t[:, dt, :], wv_in[dt * P:(dt + 1) * P, :])
        _kproj_half(1, dt_outer=False)   # kbounce-B store on the scalar ring
        for dt in range(DT):
            nc.scalar.dma_start(xqt[:, dt, :], xqt_in[dt * P:(dt + 1) * P, :])

        # V_loc[kt, e] = sum_d X_loc^T[d, kt].T Wv[d, e] (bf16), e-halves.
        _vproj_echalf(0)        # vbounce-A on sync ring (after wq)
        _vproj_echalf(1)        # vbounce-B on scalar ring (after xqt)

        # Gather outputs are replica-rank ordered = global key order on both
        # cores of a pair, so all these loads are variant-independent.
        # They go on the GPSIMD software DGE (its own descriptor ring):
        # the 8 HW-DGE rings are SHARED between the sync and scalar queue
        # engines, so a gather-gated load descriptor parked on a HW ring
        # blocks, via ring-credit waits, unrelated later stores (measured:
        # the vbounce store stuck 46us behind a kT load).  Batched into a
        # few strided descriptors since software-DGE issue is slower.
        # 4 descriptors in ascending global-key order: the scores strips
        # consume key tiles ascending, so strip 0 isn't gated on the tail
        # of a single monolithic 2MB transfer.
        for r in range(2):
            for kc in range(2):
                nc.gpsimd.dma_start(
                    kT[:, :, r * SLOC + kc * 512:r * SLOC + (kc + 1) * 512],
                    kgather[r * D:(r + 1) * D,
                            kc * 512:(kc + 1) * 512].rearrange(
                        "(et p) k -> p et k", p=P))
        # vgathers[0] (ec=0) loads in 4-tile groups, ascending, so early AV
        # slots' tiles land first.  The ec=1 loads are emitted between AV
        # pass 1 and pass 2 (see Phase B).
        for g in range(4):
            nc.gpsimd.dma_start(
                v_sb[:, 4 * g:4 * g + 4, 0:512],
                vgathers[0][4 * g * P:(4 * g + 4) * P, :].rearrange(
                    "(kt p) e -> p kt e", p=P))

        # Q^T[et, q] = sum_d Wq[d, et].T Xq^T[d, q], evicted to fp8.
        # dt-inner: the last groups' evictions don't bunch up at the end,
        # so Phase B's first strips aren't eviction-gated.
        for half in range(2):
            groups = [(et, qc) for et in range(half * 4, half * 4 + 4)
                      for qc in range(2)]
            pss = [psA.tile([P, 512], F32, tag="ps", name="ps")
                   for _ in groups]
            for gi, (et, qc) in enumerate(groups):
                for dt in range(DT):
                    nc.tensor.matmul(
                        pss[gi][:], lhsT=wq_t[:, dt, et * P:(et + 1) * P],
                        rhs=xqt[:, dt, qc * 512:(qc + 1) * 512],
                        start=(dt == 0), stop=(dt == DT - 1))
                nc.vector.tensor_copy(
                    qT[:, et, qc * 512:(qc + 1) * 512], pss[gi][:])

    # ---------------- Phase B : attention (transposed scores) ----------
    with ExitStack() as pb:
        ptpool = pb.enter_context(tc.tile_pool(name="pt", bufs=1))
        opool = pb.enter_context(tc.tile_pool(name="o", bufs=2))
        stpool = pb.enter_context(tc.tile_pool(name="stat", bufs=NSLOT))
        psS = pb.enter_context(tc.tile_pool(name="psS", bufs=2, space="PSUM"))
        psAV = pb.enter_context(tc.tile_pool(name="psAV", bufs=2, space="PSUM"))
        psRS = pb.enter_context(tc.tile_pool(name="psRS", bufs=2, space="PSUM"))

        # One flat P^T tile [k%128, kt, q]: each strip's exp is then a
        # SINGLE ScalarE instruction (~250ns fixed cost per instruction
        # dominates chunked exps).  Slot t's column range [128t, 128t+64)
        # of key tile 2t+1 is causally dead (never written) -> zero once.
        pT = ptpool.tile([P, ST, QLOC], BF16, tag="pT")
        for t in range(NSLOT):
            nc.vector.memset(pT[:, 2 * t + 1, 128 * t:128 * t + 64], 0.0)

        recips = []

        def _emit_av(t, ec):
            # One output-column pass of slot t's AV: only needs
            # v_sb[:, :, 512ec:512ec+512], i.e. V gather ec alone.
            ck = CNT[t]
            psav = psAV.tile([P, 512], F32, tag="psAV", name="psav")
            psrs = psRS.tile([P, 1], F32, tag="psRS", name="psrs") \
                if ec == 0 else None
            for kt in range(ck):
                lhsT = pT[:, kt, t * P:(t + 1) * P]
                nc.tensor.matmul(
                    psav[:], lhsT=lhsT,
                    rhs=v_sb[:, kt, ec * 512:(ec + 1) * 512],
                    start=(kt == 0), stop=(kt == ck - 1))
                if ec == 0:
                    nc.tensor.matmul(psrs[:], lhsT=lhsT, rhs=ones[:],
                                     start=(kt == 0), stop=(kt == ck - 1))
            if ec == 0:
                recip = stpool.tile([P, 1], F32, tag="rc", name="recip")
                nc.vector.reciprocal(recip[:], psrs[:])
                recips.append(recip)
            o_t = opool.tile([P, 512], F32, tag="o", name="o_t")
            nc.scalar.activation(o_t[:], psav[:],
                                 mybir.ActivationFunctionType.Copy,
                                 scale=recips[t][:])
            nc.sync.dma_start(
                out[t * P:(t + 1) * P, ec * 512:(ec + 1) * 512], o_t[:])

        for kt in range(ST):
            w = 64 * (ST - kt)              # strip covers columns [64kt, 1024)
            ps = psS.tile([P, 8 * P], F32, tag="psS", name="ps")[:, :w]
            for c0, cw in _chunks(w):
                # DoubleRow: contraction 256 per matmul via et-pair 3D APs.
                for g in range(4):
                    nc.tensor.matmul(
                        ps[:, c0:c0 + cw],
                        lhsT=kT[:, 2 * g:2 * g + 2, kt * P:(kt + 1) * P],
                        rhs=qT[:, 2 * g:2 * g + 2,
                               64 * kt + c0:64 * kt + c0 + cw],
                        start=(g == 0), stop=(g == 3), perf_mode=DR)
            # Diagonal chunk (first 64 cols): causal mask, in-place in PSUM.
            nc.vector.tensor_tensor(ps[:, 0:64], ps[:, 0:64], mask[:],
                                    op=mybir.AluOpType.add)
            # exp((scores)/32) straight from PSUM, one instruction per strip.
            nc.scalar.activation(
                pT[:, kt, 64 * kt:QLOC], ps[:, :w],
                mybir.ActivationFunctionType.Exp, scale=1.0 / 32.0)
            # AV pass 1 (output cols 0-511 + row-sums) interleaved: the
            # strip pipeline is latency-bound (matmul->mask->exp->release,
            # ~1.2us/strip with 2 PSUM strips in flight), and these fill
            # the PE bubbles.
            if kt % 2 == 1:
                _emit_av((kt - 1) // 2, 0)

        # ec=1 v_sb loads: gpsimd software DGE (see Phase A comment), in
        # 4-tile ascending groups; nothing later shares that ring.
        for g in range(4):
            nc.gpsimd.dma_start(
                v_sb[:, 4 * g:4 * g + 4, 512:1024],
                vgathers[1][4 * g * P:(4 * g + 4) * P, :].rearrange(
                    "(kt p) e -> p kt e", p=P))

        # AV pass 2 (output cols 512-1023), reusing pass 1's reciprocals.
        for t in range(NSLOT):
            _emit_av(t, 1)


_COMPILED = None


def _get_compiled():
    global _COMPILED
    if _COMPILED is None:
        _COMPILED = _build()
    return _COMPILED


def _qrows(h):
    # core column x -> global query row 128*(x//64) + 64*h + x%64
    return np.concatenate(
        [np.arange(128 * p + 64 * h, 128 * p + 64 * h + 64)
         for p in range(QLOC // 64)])


def _host_mask(h):
    # Diagonal-chunk mask, identical for every key tile kt: key r (within
    # tile) vs column j of the chunk at global row 128kt + 64h + j.
    r = np.arange(P)[:, None]
    j = np.arange(64)[None, :]
    m = np.where(r > j + 64 * h, np.float32(NEG), np.float32(0.0))
    return m.astype(ml_dtypes.bfloat16)


def _host_in_maps(X, Wq, Wk, Wv):
    bf = ml_dtypes.bfloat16
    X = np.asarray(X, np.float32)
    wq = np.asarray(Wq, np.float32).astype(bf)   # 1/sqrt(D) folded into exp
    wk = np.asarray(Wk, np.float32).astype(bf)
    wv = np.asarray(Wv, np.float32).astype(bf)
    masks = {0: _host_mask(0), 1: _host_mask(1)}
    qr = {0: _qrows(0), 1: _qrows(1)}
    in_maps = []
    for c in range(NCORES):
        b, h = divmod(c, 2)
        Xb = X[b]
        in_maps.append({
            "xt": np.ascontiguousarray(Xb[h * SLOC:(h + 1) * SLOC].T).astype(bf),
            "xqt": np.ascontiguousarray(Xb[qr[h]].T).astype(bf),
            "wq": wq, "wk": wk, "wv": wv,
            "mask": masks[h],
        })
    return in_maps, qr


def kernel(X, Wq, Wk, Wv, _trace=False):
    nc = _get_compiled()
    in_maps, qr = _host_in_maps(X, Wq, Wk, Wv)
    res = run_bass_kernel_spmd(nc, in_maps, core_ids=list(range(NCORES)),
                               trace=_trace)
    O = np.empty((B, S, D), np.float32)
    for c in range(NCORES):
        b, h = divmod(c, 2)
        O[b, qr[h]] = res.results[c]["out"]
    if _trace:
        kernel._last_exec_time_ns = res.exec_time_ns
        kernel._last_results = res
    return O


# revision 27
# speedup vs baseline: 1.1513x; 1.1513x over previous
"""Causal attention (B=4, S=2048, D=1024, fp32 in/out) on 8 Trainium2 cores.

Sharding: core c = (batch b = c//2, variant h = c%2). Queries are split at
64-row granularity: global 64-row chunk g (g=0..31 per batch) goes to
variant g%2, laid out in ascending order, so core column x maps to global
query row 128*(x//64) + 64*h + x%64.

This interleave makes the kernel's causal structure variant-INDEPENDENT:
  * scores strip for key tile kt covers exactly columns [64*kt, 1024) --
    68 (128x128x1024) tile-equivalents per core, the tile-granular minimum.
  * AV "slot" t = columns [128*t, 128*t+128) needs key tiles 0..2t+1
    (CNT = 2t+2 for every core) -- 72 tile-equivalents.
  * the causal mask reduces to ONE kt-independent [128, 64] tile applied to
    the first 64 columns of every strip (the diagonal chunk).

Numerics: Q^T and K^T are stored as fp8 e4m3 (q/k values are O(1): std
0.58, |x| < 6, well inside e4m3 range; quantization adds ~4% rms score
noise -> ~1.2% output error, comfortably under the 2e-2 budget) and the
scores matmul runs in DoubleRow perf mode: contraction 256 per matmul via
et-pair 3D APs, halving scores PE time.  The softmax 1/sqrt(D) scaling
moves into the exp (ScalarE activation scale=1/32) so q stays O(1) for
fp8.  V / P^T / AV stay bf16 (fp8 V would put ~2.6% directly on the
output).  Projections run in bf16; PSUM accumulation is fp32 throughout.

K/V are not recomputed per core: core (b, h) projects K^T/V only for its
own key half, and the pair exchanges halves with AllGathers over replica
groups [[0,1],[2,3],[4,5],[6,7]] through DRAM bounce buffers.  The CC
stream is the scarce resource (its init barrier alone costs ~40us of
kernel time and each 1MB gather ~11-16us), so: a tiny warm-up collective
fires as the FIRST gpsimd instruction to absorb the barrier, gathers are
split in halves triggered straight after each projection half's bounce
store (fp8 halves the K bytes), and the DMA rings are ordered so a bounce
store never queues behind low-urgency input loads.

Phase B per key tile kt: S^T strip [128 keys x (16-kt)*64 q] accumulated
over 4 DoubleRow e-pairs in PSUM; DVE adds the diagonal mask in-place in
PSUM; ScalarE exps straight from PSUM (scale=1/32) into per-slot bf16 P^T
tiles.  After strip 2t+1, slot t's AV runs: O = P^T.T V accumulated over
its 2t+2 key tiles with a fused ones-matmul row-sum, normalized by
1/rowsum on eviction (ScalarE Copy with scale=AP).

No max-subtraction in softmax: logits/32 have std ~0.33; masked entries
get -3.2e5 (pre-scale) -> exp underflows to 0.  ~24 dummy matmuls at t=0
warm the PE HAM clock gate while the input DMAs stream.
"""

import numpy as np
from contextlib import ExitStack

import ml_dtypes

import concourse.bass as bass
import concourse.tile as tile
from concourse import bacc, mybir
from concourse.bass_utils import run_bass_kernel_spmd

P = 128
B, S, D = 4, 2048, 1024
NCORES = 8
DT = D // P      # 8 contraction tiles
ST = S // P      # 16 key tiles (global)
SLOC = S // 2    # 1024 local keys per core
ET = D // P      # 8 output-feature tiles
QLOC = 1024      # query rows per core
NSLOT = 8        # AV slots of 128 query columns
CNT = tuple(2 * t + 2 for t in range(NSLOT))   # key tiles per slot
NEG = -320000.0  # pre-scale mask; /32 in the exp -> -1e4
NWARM = 14       # HAM warm-up matmuls

F32 = mybir.dt.float32
BF16 = mybir.dt.bfloat16
FP8 = mybir.dt.float8e4
FP8V = mybir.dt.float8e3   # e3m4 V exchange: 4-bit mantissa, range +-15.5, exact in bf16
DR = mybir.MatmulPerfMode.DoubleRow

REPLICA_GROUPS = [[0, 1], [2, 3], [4, 5], [6, 7]]


def _chunks(width, step=512):
    out = []
    c0 = 0
    while c0 < width:
        out.append((c0, min(step, width - c0)))
        c0 += out[-1][1]
    return out


def _build():
    nc = bacc.Bacc("TRN2", target_bir_lowering=False, debug=False,
                   num_devices=NCORES)
    xt_in = nc.dram_tensor("xt", [D, SLOC], BF16, kind="ExternalInput").ap()
    xqt_in = nc.dram_tensor("xqt", [D, QLOC], BF16, kind="ExternalInput").ap()
    wq_in = nc.dram_tensor("wq", [D, D], BF16, kind="ExternalInput").ap()
    wk_in = nc.dram_tensor("wk", [D, D], BF16, kind="ExternalInput").ap()
    wv_in = nc.dram_tensor("wv", [D, D], BF16, kind="ExternalInput").ap()
    mask_in = nc.dram_tensor("mask", [P, 64], BF16, kind="ExternalInput").ap()
    out = nc.dram_tensor("out", [QLOC, D], F32, kind="ExternalOutput").ap()

    with tile.TileContext(nc) as tc, ExitStack() as ctx:
        persist = ctx.enter_context(tc.tile_pool(name="persist", bufs=1))
        kT = persist.tile([P, ET, S], FP8, tag="kT")       # K^T [e%128, et, key]
        qT = persist.tile([P, ET, QLOC], FP8, tag="qT")    # Q^T [e%128, et, q]
        v_sb = persist.tile([P, ST, D], BF16, tag="v")     # V   [k%128, kt, e]
        # e3m4 staging for the gathered V halves; DVE upcasts into v_sb
        # (exact), since the software-DGE cast-DMA measured far too slow.
        v8s = [persist.tile([P, ST, 512], mybir.dt.float8e3, tag=f"v8_{i}",
                            name=f"v8_{i}") for i in range(2)]
        ones = persist.tile([P, 1], BF16, tag="ones")
        mask = persist.tile([P, 64], BF16, tag="mask")
        warm = persist.tile([P, 512], BF16, tag="warm")
        nc.gpsimd.memset(ones[:], 1.0)
        nc.gpsimd.memset(warm[:], 0.25)

        _emit_body(nc, tc, xt_in, xqt_in, wq_in, wk_in, wv_in, mask_in, out,
                   kT, qT, v_sb, v8s, ones, mask, warm)
    nc.compile()
    return nc


def _emit_body(nc, tc, xt_in, xqt_in, wq_in, wk_in, wv_in, mask_in, out,
               kT, qT, v_sb, v8s, ones, mask, warm):
    # ---------------- Phase A : projections + KV exchange ----------------
    with ExitStack() as pa:
        xp = pa.enter_context(tc.tile_pool(name="xp", bufs=1))
        dp = pa.enter_context(tc.tile_pool(name="dp", bufs=1, space="DRAM"))
        psA = pa.enter_context(tc.tile_pool(name="psA", bufs=8, space="PSUM"))

        # Tiny warm-up collective FIRST on the gpsimd queue: the CC stream's
        # init barrier (an all-core rendezvous, ~20us after ~15us of core
        # skew) starts at the first trigger, so trigger it at t~1us.
        warm_in = dp.tile([P, 8], BF16, tag="warm_in")
        warm_out = dp.tile([2 * P, 8], BF16, tag="warm_out")
        nc.gpsimd.collective_compute(
            "AllGather", mybir.AluOpType.bypass,
            replica_groups=REPLICA_GROUPS,
            ins=[warm_in.opt()], outs=[warm_out.opt()])

        # PE warm-up: keeps the HAM activity window busy from t~1us so the
        # first real matmul (waiting on wk/xt DMA) already runs at 2.4GHz.
        psw = psA.tile([P, 512], F32, tag="ps", name="psw")
        for _ in range(NWARM):
            nc.tensor.matmul(psw[:], lhsT=warm[:, 0:P], rhs=warm[:],
                             start=True, stop=True)

        xt = xp.tile([P, DT, SLOC], BF16, tag="xt")
        wq_t = xp.tile([P, DT, D], BF16, tag="wq")
        wk_t = xp.tile([P, DT, D], BF16, tag="wk")
        wv_t = xp.tile([P, DT, D], BF16, tag="wv")
        xqt = xp.tile([P, DT, QLOC], BF16, tag="xqt")
        klocal = xp.tile([P, ET, SLOC], FP8, tag="klocal")
        vlocal = xp.tile([P, ST // 2, D], FP8V, tag="vlocal")
        kbounce = dp.tile([D, SLOC], FP8, tag="kbounce")
        kgather = dp.tile([2 * D, SLOC], FP8, tag="kgather")
        # V is exchanged split by OUTPUT COLUMNS (e-halves), not key tiles:
        # AV output cols [0,512) need only gather A, cols [512,1024) only
        # gather B, so the AV phase splits into two column passes that track
        # the two gathers' arrival with no stall.
        vbounces = [dp.tile([SLOC, 512], FP8V, tag=f"vbounce{i}",
                            name=f"vbounce{i}") for i in range(2)]
        vgathers = [dp.tile([S, 512], FP8V, tag=f"vgather{i}",
                            name=f"vgather{i}") for i in range(2)]

        # Input DMAs.  K proj needs wk+xt slices in dt order first: stream
        # them on the two HW-DGE rings in parallel; the rest follow in
        # urgency order.  The bounce stores are emitted between input loads
        # at the points where their data is ready, so they never wait
        # behind a low-urgency input transfer.
        for dt in range(DT):
            nc.sync.dma_start(wk_t[:, dt, :], wk_in[dt * P:(dt + 1) * P, :])
            nc.scalar.dma_start(xt[:, dt, :], xt_in[dt * P:(dt + 1) * P, :])
        nc.gpsimd.dma_start(mask[:], mask_in[:, :])

        def _kproj_half(half, dt_outer):
            groups = [(et, kc) for et in range(half * 4, half * 4 + 4)
                      for kc in range(2)]
            pss = [psA.tile([P, 512], F32, tag="ps", name="ps")
                   for _ in groups]
            if dt_outer:
                # matmuls start as soon as the first wk/xt slices land
                for dt in range(DT):
                    for gi, (et, kc) in enumerate(groups):
                        nc.tensor.matmul(
                            pss[gi][:], lhsT=wk_t[:, dt, et * P:(et + 1) * P],
                            rhs=xt[:, dt, kc * 512:(kc + 1) * 512],
                            start=(dt == 0), stop=(dt == DT - 1))
                for gi, (et, kc) in enumerate(groups):
                    nc.vector.tensor_copy(
                        klocal[:, et, kc * 512:(kc + 1) * 512], pss[gi][:])
            else:
                # dt-inner: each group finishes early so its eviction
                # overlaps the remaining groups' matmuls.
                for gi, (et, kc) in enumerate(groups):
                    for dt in range(DT):
                        nc.tensor.matmul(
                            pss[gi][:], lhsT=wk_t[:, dt, et * P:(et + 1) * P],
                            rhs=xt[:, dt, kc * 512:(kc + 1) * 512],
                            start=(dt == 0), stop=(dt == DT - 1))
                    nc.vector.tensor_copy(
                        klocal[:, et, kc * 512:(kc + 1) * 512], pss[gi][:])
            et0 = half * 4
            qeng = nc.sync if half == 0 else nc.scalar
            qeng.dma_start(
                kbounce[et0 * P:(et0 + 4) * P, :].rearrange(
                    "(et p) k -> p et k", p=P),
                klocal[:, et0:et0 + 4, :])
            if half == 1:
                # Single 1MB fp8 gather for all of K: fewer ops on the
                # serial CC stream pulls both V gathers earlier.
                nc.gpsimd.collective_compute(
                    "AllGather", mybir.AluOpType.bypass,
                    replica_groups=REPLICA_GROUPS,
                    ins=[kbounce.opt()], outs=[kgather.opt()])

        def _vproj_echalf(ec):
            # One e-column half of V for ALL 8 local key tiles: 8 groups,
            # dt-inner so evictions (and the bounce store) pipeline.
            pss = [psA.tile([P, 512], F32, tag="ps", name="ps")
                   for _ in range(8)]
            for st in range(8):
                for dt in range(DT):
                    nc.tensor.matmul(
                        pss[st][:], lhsT=xt[:, dt, st * P:(st + 1) * P],
                        rhs=wv_t[:, dt, ec * 512:(ec + 1) * 512],
                        start=(dt == 0), stop=(dt == DT - 1))
                nc.vector.tensor_copy(
                    vlocal[:, st, ec * 512:(ec + 1) * 512], pss[st][:])
            qeng = nc.sync if ec == 0 else nc.scalar
            qeng.dma_start(
                vbounces[ec].rearrange("(st p) e -> p st e", p=P),
                vlocal[:, :, ec * 512:(ec + 1) * 512])
            nc.gpsimd.collective_compute(
                "AllGather", mybir.AluOpType.bypass,
                replica_groups=REPLICA_GROUPS,
                ins=[vbounces[ec].opt()], outs=[vgathers[ec].opt()])

        # K^T_loc[et, k] = sum_d Wk[d, et].T X_loc^T[d, k], evicted to fp8.
        _kproj_half(0, dt_outer=True)
        # sync ring: kbounce-A store just went in; wq follows (Q proj needs
        # it only at ~70us).  scalar ring: wv next (V proj needs it at ~40).
        for dt in range(DT):
            nc.sync.dma_start(wq_t[:, dt, :], wq_in[dt * P:(dt + 1) * P, :])
            nc.scalar.dma_start(wv_t[:, dt, :], wv_in[dt * P:(dt + 1) * P, :])
        _kproj_half(1, dt_outer=False)   # kbounce-B store on the scalar ring
        for dt in range(DT):
            nc.scalar.dma_start(xqt[:, dt, :], xqt_in[dt * P:(dt + 1) * P, :])

        # V_loc[kt, e] = sum_d X_loc^T[d, kt].T Wv[d, e] (bf16), e-halves.
        _vproj_echalf(0)        # vbounce-A on sync ring (after wq)
        _vproj_echalf(1)        # vbounce-B on scalar ring (after xqt)

        # Gather outputs are replica-rank ordered = global key order on both
        # cores of a pair, so all these loads are variant-independent.
        # They go on the GPSIMD software DGE (its own descriptor ring):
        # the 8 HW-DGE rings are SHARED between the sync and scalar queue
        # engines, so a gather-gated load descriptor parked on a HW ring
        # blocks, via ring-credit waits, unrelated later stores (measured:
        # the vbounce store stuck 46us behind a kT load).  Batched into a
        # few strided descriptors since software-DGE issue is slower.
        # 4 descriptors in ascending global-key order: the scores strips
        # consume key tiles ascending, so strip 0 isn't gated on the tail
        # of a single monolithic 2MB transfer.
        for r in range(2):
            for kc in range(2):
                nc.gpsimd.dma_start(
                    kT[:, :, r * SLOC + kc * 512:r * SLOC + (kc + 1) * 512],
                    kgather[r * D:(r + 1) * D,
                            kc * 512:(kc + 1) * 512].rearrange(
                        "(et p) k -> p et k", p=P))
        # vgathers[0] (ec=0) raw fp8 loads in 4-tile groups, ascending, so
        # early AV slots' tiles land first.  The DVE upcasts into v_sb are
        # emitted early in Phase B (their DVE-FIFO slots must not block the
        # strip mask-adds, and the loads land before Phase B starts).  The
        # ec=1 loads are emitted between AV pass 1 and pass 2.
        for g in range(4):
            nc.gpsimd.dma_start(
                v8s[0][:, 4 * g:4 * g + 4, :],
                vgathers[0][4 * g * P:(4 * g + 4) * P, :].rearrange(
                    "(kt p) e -> p kt e", p=P))

        # Q^T[et, q] = sum_d Wq[d, et].T Xq^T[d, q], evicted to fp8.
        # dt-inner: the last groups' evictions don't bunch up at the end,
        # so Phase B's first strips aren't eviction-gated.
        for half in range(2):
            groups = [(et, qc) for et in range(half * 4, half * 4 + 4)
                      for qc in range(2)]
            pss = [psA.tile([P, 512], F32, tag="ps", name="ps")
                   for _ in groups]
            for gi, (et, qc) in enumerate(groups):
                for dt in range(DT):
                    nc.tensor.matmul(
                        pss[gi][:], lhsT=wq_t[:, dt, et * P:(et + 1) * P],
                        rhs=xqt[:, dt, qc * 512:(qc + 1) * 512],
                        start=(dt == 0), stop=(dt == DT - 1))
                nc.vector.tensor_copy(
                    qT[:, et, qc * 512:(qc + 1) * 512], pss[gi][:])

    # ---------------- Phase B : attention (transposed scores) ----------
    with ExitStack() as pb:
        ptpool = pb.enter_context(tc.tile_pool(name="pt", bufs=1))
        opool = pb.enter_context(tc.tile_pool(name="o", bufs=2))
        stpool = pb.enter_context(tc.tile_pool(name="stat", bufs=NSLOT))
        psS = pb.enter_context(tc.tile_pool(name="psS", bufs=2, space="PSUM"))
        psAV = pb.enter_context(tc.tile_pool(name="psAV", bufs=2, space="PSUM"))
        psRS = pb.enter_context(tc.tile_pool(name="psRS", bufs=2, space="PSUM"))

        # One flat P^T tile [k%128, kt, q]: each strip's exp is then a
        # SINGLE ScalarE instruction (~250ns fixed cost per instruction
        # dominates chunked exps).  Slot t's column range [128t, 128t+64)
        # of key tile 2t+1 is causally dead (never written) -> zero once.
        pT = ptpool.tile([P, ST, QLOC], BF16, tag="pT")
        for t in range(NSLOT):
            nc.vector.memset(pT[:, 2 * t + 1, 128 * t:128 * t + 64], 0.0)
        # ec=0 V upcasts (loads have landed by now; ~1.5us each on DVE,
        # ahead of the strip mask-adds but those aren't needed for ~2us)
        for g in range(4):
            nc.vector.tensor_copy(v_sb[:, 4 * g:4 * g + 4, 0:512],
                                  v8s[0][:, 4 * g:4 * g + 4, :])

        recips = []

        def _emit_av(t, ec):
            # One output-column pass of slot t's AV: only needs
            # v_sb[:, :, 512ec:512ec+512], i.e. V gather ec alone.
            ck = CNT[t]
            psav = psAV.tile([P, 512], F32, tag="psAV", name="psav")
            psrs = psRS.tile([P, 1], F32, tag="psRS", name="psrs") \
                if ec == 0 else None
            for kt in range(ck):
                lhsT = pT[:, kt, t * P:(t + 1) * P]
                nc.tensor.matmul(
                    psav[:], lhsT=lhsT,
                    rhs=v_sb[:, kt, ec * 512:(ec + 1) * 512],
                    start=(kt == 0), stop=(kt == ck - 1))
                if ec == 0:
                    nc.tensor.matmul(psrs[:], lhsT=lhsT, rhs=ones[:],
                                     start=(kt == 0), stop=(kt == ck - 1))
            if ec == 0:
                recip = stpool.tile([P, 1], F32, tag="rc", name="recip")
                nc.vector.reciprocal(recip[:], psrs[:])
                recips.append(recip)
            o_t = opool.tile([P, 512], F32, tag="o", name="o_t")
            nc.scalar.activation(o_t[:], psav[:],
                                 mybir.ActivationFunctionType.Copy,
                                 scale=recips[t][:])
            nc.sync.dma_start(
                out[t * P:(t + 1) * P, ec * 512:(ec + 1) * 512], o_t[:])

        for kt in range(ST):
            w = 64 * (ST - kt)              # strip covers columns [64kt, 1024)
            ps = psS.tile([P, 8 * P], F32, tag="psS", name="ps")[:, :w]
            for c0, cw in _chunks(w):
                # DoubleRow: contraction 256 per matmul via et-pair 3D APs.
                for g in range(4):
                    nc.tensor.matmul(
                        ps[:, c0:c0 + cw],
                        lhsT=kT[:, 2 * g:2 * g + 2, kt * P:(kt + 1) * P],
                        rhs=qT[:, 2 * g:2 * g + 2,
                               64 * kt + c0:64 * kt + c0 + cw],
                        start=(g == 0), stop=(g == 3), perf_mode=DR)
            # Diagonal chunk (first 64 cols): causal mask, in-place in PSUM.
            nc.vector.tensor_tensor(ps[:, 0:64], ps[:, 0:64], mask[:],
                                    op=mybir.AluOpType.add)
            # exp((scores)/32) straight from PSUM, one instruction per strip.
            nc.scalar.activation(
                pT[:, kt, 64 * kt:QLOC], ps[:, :w],
                mybir.ActivationFunctionType.Exp, scale=1.0 / 32.0)
            # AV pass 1 (output cols 0-511 + row-sums) interleaved: the
            # strip pipeline is latency-bound (matmul->mask->exp->release,
            # ~1.2us/strip with 2 PSUM strips in flight), and these fill
            # the PE bubbles.
            if kt % 2 == 1:
                _emit_av((kt - 1) // 2, 0)

        # ec=1 v_sb loads: gpsimd software DGE (see Phase A comment), in
        # 4-tile ascending groups + DVE upcasts; nothing later uses either
        # queue, so the gather-gated waits block nothing.
        for g in range(4):
            nc.gpsimd.dma_start(
                v8s[1][:, 4 * g:4 * g + 4, :],
                vgathers[1][4 * g * P:(4 * g + 4) * P, :].rearrange(
                    "(kt p) e -> p kt e", p=P))
            nc.vector.tensor_copy(v_sb[:, 4 * g:4 * g + 4, 512:1024],
                                  v8s[1][:, 4 * g:4 * g + 4, :])

        # AV pass 2 (output cols 512-1023), reusing pass 1's reciprocals.
        for t in range(NSLOT):
            _emit_av(t, 1)


_COMPILED = None


def _get_compiled():
    global _COMPILED
    if _COMPILED is None:
        _COMPILED = _build()
    return _COMPILED


def _qrows(h):
    # core column x -> global query row 128*(x//64) + 64*h + x%64
    return np.concatenate(
        [np.arange(128 * p + 64 * h, 128 * p + 64 * h + 64)
         for p in range(QLOC // 64)])


def _host_mask(h):
    # Diagonal-chunk mask, identical for every key tile kt: key r (within
    # tile) vs column j of the chunk at global row 128kt + 64h + j.
    r = np.arange(P)[:, None]
    j = np.arange(64)[None, :]
    m = np.where(r > j + 64 * h, np.float32(NEG), np.float32(0.0))
    return m.astype(ml_dtypes.bfloat16)


def _host_in_maps(X, Wq, Wk, Wv):
    bf = ml_dtypes.bfloat16
    X = np.asarray(X, np.float32)
    wq = np.asarray(Wq, np.float32).astype(bf)   # 1/sqrt(D) folded into exp
    wk = np.asarray(Wk, np.float32).astype(bf)
    wv = np.asarray(Wv, np.float32).astype(bf)
    masks = {0: _host_mask(0), 1: _host_mask(1)}
    qr = {0: _qrows(0), 1: _qrows(1)}
    in_maps = []
    for c in range(NCORES):
        b, h = divmod(c, 2)
        Xb = X[b]
        in_maps.append({
            "xt": np.ascontiguousarray(Xb[h * SLOC:(h + 1) * SLOC].T).astype(bf),
            "xqt": np.ascontiguousarray(Xb[qr[h]].T).astype(bf),
            "wq": wq, "wk": wk, "wv": wv,
            "mask": masks[h],
        })
    return in_maps, qr


def kernel(X, Wq, Wk, Wv, _trace=False):
    nc = _get_compiled()
    in_maps, qr = _host_in_maps(X, Wq, Wk, Wv)
    res = run_bass_kernel_spmd(nc, in_maps, core_ids=list(range(NCORES)),
                               trace=_trace)
    O = np.empty((B, S, D), np.float32)
    for c in range(NCORES):
        b, h = divmod(c, 2)
        O[b, qr[h]] = res.results[c]["out"]
    if _trace:
        kernel._last_exec_time_ns = res.exec_time_ns
        kernel._last_results = res
    return O
